# revision 8
# baseline (speedup 1.0000x reference)
"""CompressedLinear on 8 Trainium2 NeuronCores.

out[b,s,o] = sum_i x[b,s,i] * (w_int8[o,i] * scale[o]) + bias[o]
  x: [4, 2048, 4096] f32, w_int8: [16384, 4096] int32 (codes in [-64,63]),
  scale/bias: [16384] f32 -> out: [4, 2048, 16384] f32

Strategy (tensor-parallel over out_features):
  - Each of the 8 cores owns a 2048-row slice of W/scale/bias and computes
    out[:, :, c*2048:(c+1)*2048]; x is replicated.
  - Weights codes are exact in bf16; scale is applied AFTER the matmul
    (per-out-feature), so the matmul itself is integer-exact in bf16.
  - x (f32) is rounded to bf16 host-side; a single bf16 matmul pass
    accumulates in f32 PSUM -> ~1e-3 relative error (tolerance is 2e-2)
    at 1x bf16 matmul cost. This is the PE roofline: fp8 DoubleRow would
    need >=3 digit-product passes at 2x rate for this precision (worse),
    and f32 runs at 1/4 rate.
  - Per core loop: stationary operand = 128-token column block of x^T,
    moving operand = w^T; PSUM holds [128 tokens, 4x512 outfeat]; 32 k-tiles
    x 4 banks = 128 matmuls per token tile, then one fused
    scale-mult + bias-add epilogue on DVE and a DMA store.

All data layout transforms (transpose, hi/lo split, int8->bf16 cast,
scale/bias broadcast) are host-side numpy; gather is a concat.
"""

import os

import numpy as np
import ml_dtypes

BF16 = ml_dtypes.bfloat16

OUT, IN = 16384, 4096
B, S = 4, 2048
TOK = B * S            # 8192 tokens
NCORES = 8
OSH = OUT // NCORES    # 2048 out-features per core
KT = IN // 128         # 32 k-tiles
TT = TOK // 128        # 64 token tiles
NB = OSH // 512        # 4 psum banks per token tile

_last_results = None   # BassKernelResults of the most recent run (for test.py)


def _build_program():
    from contextlib import ExitStack

    import concourse.bass as bass
    import concourse.tile as tile
    from concourse import mybir

    f32 = mybir.dt.float32
    bf16 = mybir.dt.bfloat16

    nc = bass.Bass()
    xhi_d = nc.declare_dram_parameter("xhi", [TT, 128, KT, 128], bf16, isOutput=False)
    w_d = nc.declare_dram_parameter("w", [128, KT, OSH], bf16, isOutput=False)
    scale_d = nc.declare_dram_parameter("scale", [128, NB, 512], f32, isOutput=False)
    bias_d = nc.declare_dram_parameter("bias", [128, NB, 512], f32, isOutput=False)
    out_d = nc.declare_dram_parameter("out", [TT, 128, NB, 512], f32, isOutput=True)

    from concourse.tile import add_dep_helper

    with tile.TileContext(nc) as tc, ExitStack() as ctx:
        wpool = ctx.enter_context(tc.tile_pool(name="w", bufs=1))
        cpool = ctx.enter_context(tc.tile_pool(name="consts", bufs=1))
        xpool = ctx.enter_context(tc.tile_pool(name="x", bufs=2))
        opool = ctx.enter_context(tc.tile_pool(name="o", bufs=2))
        pspool = ctx.enter_context(tc.tile_pool(name="ps", bufs=2, space="PSUM"))

        w_sb = wpool.tile([128, KT, OSH], bf16)
        w_dma = nc.sync.dma_start(w_sb[:], w_d[:])
        scale_sb = cpool.tile([128, NB, 512], f32, tag="scale")
        scale_dma = nc.sync.dma_start(scale_sb[:], scale_d[:])
        bias_sb = cpool.tile([128, NB, 512], f32, tag="bias")
        bias_dma = nc.sync.dma_start(bias_sb[:], bias_d[:])

        # Per-iteration disjoint scratch columns -> the carrier ops carry no
        # WAW deps of their own.
        scratch = cpool.tile([1, TT], f32, tag="scratch")
        dummy = cpool.tile([1, 3 * TT], f32, tag="dummy")
        dveA = cpool.tile([1, TT], f32, tag="dveA")
        dveB = cpool.tile([1, TT], f32, tag="dveB")
        # Preamble DVE carriers: observe the scale/bias const loads on DVE so
        # no steady-state DVE op pairs a DMAHW wait with another wait.
        pre = cpool.tile([1, 2], f32, tag="pre")
        nc.vector.tensor_copy(pre[:, 0:1], scale_sb[:1, 0, :1])
        nc.vector.tensor_copy(pre[:, 1:2], bias_sb[:1, 0, :1])

        psum_readers = []  # the scale-mult (last psum reader) per iteration
        last_mms = []  # final matmul per iteration
        out_dmas = []
        out_copies = []
        x_dmas = []
        adds = []

        # Hardware sync-wait slots are tiny (1 per PE LW/MM and per SWDGE
        # DMA, 2 per HWDGE DMA), and Tile's wait assignment is per-proc
        # minimal but not transitive. So every cross-engine dependency is
        # absorbed by a dedicated cheap "carrier" op on the consuming engine,
        # with explicit ordering edges so the scheduler keeps each carrier
        # ahead of its dependents and every instruction introduces at most
        # one new wait.
        def order(after, before):
            add_dep_helper(after.ins, before.ins, sync=False, reason="carrier order")

        for t in range(TT):
            xhi = xpool.tile([128, KT, 128], bf16, tag="xhi")
            # POOL carrier chain, one wait each: gen-2 x-load DMA (its
            # lane sem would otherwise ride the new DMA as a WAW wait) and
            # gen-2 matmul (x slot reader), before the x-slot rewrite.
            ms1 = nc.gpsimd.memset(dummy[:, 3 * t : 3 * t + 1], 0)
            ms3 = nc.gpsimd.memset(dummy[:, 3 * t + 2 : 3 * t + 3], 0)
            order(ms3, ms1)
            if t >= 2:
                add_dep_helper(
                    ms1.ins, x_dmas[t - 2].ins, reason="x WAW lane via carrier"
                )
                add_dep_helper(
                    ms3.ins,
                    last_mms[t - 2].ins,
                    reason="x slot reuse gated on POOL carrier",
                )
            d2 = nc.gpsimd.dma_start(xhi[:], xhi_d[t])
            order(d2, ms3)
            x_dmas.append(d2)

            ps = pspool.tile([128, NB, 512], f32)
            # PE carrier: guard LDWEIGHTS absorbing the psum-slot-free (DVE)
            # wait so the first real matmul only waits on PE.
            guard = nc.tensor.ldweights(w_sb[:, 0, :128])
            if t >= 2:
                add_dep_helper(
                    guard.ins,
                    psum_readers[t - 2].ins,
                    reason="psum slot reuse gated on guard ldweights",
                )
            first_mm = None
            for k in range(KT):
                for j in range(NB):
                    mm = nc.tensor.matmul(
                        ps[:, j, :],
                        xhi[:, k, :],
                        w_sb[:, k, j * 512 : (j + 1) * 512],
                        start=(k == 0),
                        stop=(k == KT - 1),
                    )
                    if first_mm is None:
                        first_mm = mm
            order(first_mm, guard)
            last_mms.append(mm)

            ob = opool.tile([128, NB, 512], f32)
            # DVE carriers: absorb the ob-slot WAR deps (gen-2 out-store DMA
            # and gen-2 POOL scratch copy) ahead of the scale-mult.
            c1 = nc.vector.tensor_copy(dveA[:, t : t + 1], scale_sb[:1, 0, :1])
            c2 = nc.vector.tensor_copy(dveB[:, t : t + 1], scale_sb[:1, 0, :1])
            if t >= 2:
                add_dep_helper(
                    c1.ins, out_dmas[t - 2].ins, reason="ob reuse vs out dma"
                )
                add_dep_helper(
                    c2.ins, out_copies[t - 2].ins, reason="ob reuse vs pool copy"
                )
            mult = nc.vector.tensor_tensor(
                ob[:], ps[:], scale_sb[:], mybir.AluOpType.mult
            )
            order(mult, c1)
            order(mult, c2)
            psum_readers.append(mult)
            adds.append(
                nc.vector.tensor_tensor(ob[:], ob[:], bias_sb[:], mybir.AluOpType.add)
            )
            # POOL carrier: RAW on ob -> absorbs the DVE wait ahead of the
            # out-store.
            cp = nc.gpsimd.tensor_copy(scratch[:, t : t + 1], ob[:1, 0, :1])
            od = nc.gpsimd.dma_start(out_d[t], ob[:])
            order(od, cp)
            out_copies.append(cp)
            out_dmas.append(od)

        # Tail carriers: SP nops, one wait each, observing every outstanding
        # sem (PE, DVE, Pool, all SWDGE lanes, preamble HWDGE lanes) so the
        # kernel-tail SP drain doesn't exceed its sync-wait slots.
        tail_deps = [
            last_mms[-1],
            adds[-1],
            out_copies[-1],
            w_dma,
            scale_dma,
            bias_dma,
        ]
        for i in (1, 2, 3):
            tail_deps += [out_dmas[-i], x_dmas[-i]]
        for i, dep in enumerate(tail_deps):
            nop = nc.engines[mybir.EngineType.SP].nop(
                nofuse=True, hint=f"tail_carrier_{i}"
            )
            add_dep_helper(nop.ins, dep.ins, reason="tail drain carrier")

    return nc


def kernel(x, weight_int8, scale, bias):
    global _last_results
    from concourse.bass_utils import run_bass_kernel_spmd

    x = np.asarray(x)
    weight_int8 = np.asarray(weight_int8)
    scale = np.asarray(scale, dtype=np.float32)
    bias = np.asarray(bias, dtype=np.float32)

    # x^T [IN, TOK] rounded to bf16, tiled to [TT, 128p(IN), KT, 128(tok)]
    xT = np.ascontiguousarray(x.reshape(TOK, IN).astype(np.float32).T)
    x_hi = xT.astype(BF16)
    x_hi = np.ascontiguousarray(
        x_hi.reshape(KT, 128, TT, 128).transpose(2, 1, 0, 3)
    )

    in_maps = []
    for c in range(NCORES):
        wc = weight_int8[c * OSH : (c + 1) * OSH].astype(np.float32).astype(BF16)
        # w^T [IN, OSH] tiled to [128p(IN), KT, OSH]
        wp = np.ascontiguousarray(wc.T.reshape(KT, 128, OSH).transpose(1, 0, 2))
        sc = np.ascontiguousarray(
            np.broadcast_to(scale[c * OSH : (c + 1) * OSH], (128, OSH))
        ).reshape(128, NB, 512)
        bc = np.ascontiguousarray(
            np.broadcast_to(bias[c * OSH : (c + 1) * OSH], (128, OSH))
        ).reshape(128, NB, 512)
        in_maps.append({"xhi": x_hi, "w": wp, "scale": sc, "bias": bc})

    nc = _build_program()
    trace = bool(os.environ.get("KERNEL_TRACE"))
    kwargs = {}
    if trace:
        # Local-only profiling: stub the bucket upload and install the axon
        # NTFF hook (the image's antenv stub lacks axon_hooks).
        import sys
        import types

        from concourse import bass_utils as _bu

        _bu.upload_artifacts = lambda tmpdir: "local://" + tmpdir
        if "antenv.axon_hooks" not in sys.modules:
            import antenv

            mod = types.ModuleType("antenv.axon_hooks")
            _holder = [None]
            mod.set_axon_ntff_profile_hook = lambda h: _holder.__setitem__(0, h)
            mod.get_axon_ntff_profile_hook = lambda: _holder[0]
            antenv.axon_hooks = mod
            sys.modules["antenv.axon_hooks"] = mod
        from antenv.axon_hooks import (
            get_axon_ntff_profile_hook,
            set_axon_ntff_profile_hook,
        )

        if get_axon_ntff_profile_hook() is None:
            from trn_agent_boot.trn_boot import _ntff_profile_via_ctypes

            set_axon_ntff_profile_hook(
                _ntff_profile_via_ctypes(
                    os.environ.get("PJRT_LIBRARY_PATH", "/opt/axon/libaxon_pjrt.so")
                )
            )
        tmpdir = os.environ.get("KERNEL_TRACE_DIR")
        if tmpdir:
            os.makedirs(tmpdir, exist_ok=True)
            kwargs["tmpdir"] = tmpdir

    res = run_bass_kernel_spmd(
        nc,
        in_maps,
        list(range(NCORES)),
        trace=trace,
        **kwargs,
    )
    _last_results = res

    parts = [res.results[c]["out"].reshape(TOK, OSH) for c in range(NCORES)]
    return np.concatenate(parts, axis=1).reshape(B, S, OUT)



# revision 9
# speedup vs baseline: 1.9587x; 1.9587x over previous
"""CompressedLinear on 8 Trainium2 NeuronCores.

out[b,s,o] = sum_i x[b,s,i] * (w_int8[o,i] * scale[o]) + bias[o]
  x: [4, 2048, 4096] f32, w_int8: [16384, 4096] int32 (codes in [-64,63]),
  scale/bias: [16384] f32 -> out: [4, 2048, 16384] f32

Strategy (tensor-parallel over out_features):
  - Each of the 8 cores owns a 2048-row slice of W/scale/bias and computes
    out[:, :, c*2048:(c+1)*2048]; x is replicated.
  - Weights codes are exact in bf16; scale is applied AFTER the matmul
    (per-out-feature), so the matmul itself is integer-exact in bf16.
  - x (f32) is rounded to bf16 host-side; a single bf16 matmul pass
    accumulates in f32 PSUM -> ~1e-3 relative error (tolerance is 2e-2)
    at 1x bf16 matmul cost. This is the PE roofline: fp8 DoubleRow would
    need >=3 digit-product passes at 2x rate for this precision (worse),
    and f32 runs at 1/4 rate.
  - Per core loop: stationary operand = 128-token column block of x^T,
    moving operand = w^T; PSUM holds [128 tokens, 4x512 outfeat]; 32 k-tiles
    x 4 banks = 128 matmuls per token tile, then one fused
    scale-mult + bias-add epilogue on DVE and a DMA store.

All data layout transforms (transpose, hi/lo split, int8->bf16 cast,
scale/bias broadcast) are host-side numpy; gather is a concat.
"""

import os

import numpy as np
import ml_dtypes

BF16 = ml_dtypes.bfloat16

OUT, IN = 16384, 4096
B, S = 4, 2048
TOK = B * S            # 8192 tokens
NCORES = 8
OSH = OUT // NCORES    # 2048 out-features per core
KT = IN // 128         # 32 k-tiles
TT = TOK // 128        # 64 token tiles
NB = OSH // 512        # 4 psum banks per token tile

_last_results = None   # BassKernelResults of the most recent run (for test.py)


def _build_program():
    from contextlib import ExitStack

    import concourse.bass as bass
    import concourse.tile as tile
    from concourse import mybir

    f32 = mybir.dt.float32
    bf16 = mybir.dt.bfloat16

    nc = bass.Bass()
    xhi_d = nc.declare_dram_parameter("xhi", [TT, 128, KT, 128], bf16, isOutput=False)
    w_d = nc.declare_dram_parameter("w", [128, KT, OSH], bf16, isOutput=False)
    scale_d = nc.declare_dram_parameter("scale", [128, NB, 512], f32, isOutput=False)
    bias_d = nc.declare_dram_parameter("bias", [128, NB, 512], f32, isOutput=False)
    out_d = nc.declare_dram_parameter("out", [TT, 128, NB, 512], f32, isOutput=True)

    from concourse.tile import add_dep_helper

    with tile.TileContext(nc) as tc, ExitStack() as ctx:
        wpool = ctx.enter_context(tc.tile_pool(name="w", bufs=1))
        cpool = ctx.enter_context(tc.tile_pool(name="consts", bufs=1))
        xpool = ctx.enter_context(tc.tile_pool(name="x", bufs=2))
        opool = ctx.enter_context(tc.tile_pool(name="o", bufs=2))
        pspool = ctx.enter_context(tc.tile_pool(name="ps", bufs=2, space="PSUM"))

        w_sb = wpool.tile([128, KT, OSH], bf16)
        w_dma = nc.sync.dma_start(w_sb[:], w_d[:])
        scale_sb = cpool.tile([128, NB, 512], f32, tag="scale")
        scale_dma = nc.sync.dma_start(scale_sb[:], scale_d[:])
        bias_sb = cpool.tile([128, NB, 512], f32, tag="bias")
        bias_dma = nc.sync.dma_start(bias_sb[:], bias_d[:])

        # Per-iteration disjoint scratch columns -> the carrier ops carry no
        # WAW deps of their own.
        scratch = cpool.tile([1, TT], f32, tag="scratch")
        dummy = cpool.tile([1, 3 * TT], f32, tag="dummy")
        dveA = cpool.tile([1, TT], f32, tag="dveA")
        dveB = cpool.tile([1, TT], f32, tag="dveB")
        # Preamble DVE carriers: observe the scale/bias const loads on DVE so
        # no steady-state DVE op pairs a DMAHW wait with another wait.
        pre = cpool.tile([1, 2], f32, tag="pre")
        nc.vector.tensor_copy(pre[:, 0:1], scale_sb[:1, 0, :1])
        nc.vector.tensor_copy(pre[:, 1:2], bias_sb[:1, 0, :1])

        psum_readers = []  # the scale-mult (last psum reader) per iteration
        last_mms = []  # final matmul per iteration
        out_dmas = []
        out_copies = []
        x_dmas = []
        adds = []

        # Hardware sync-wait slots are tiny (1 per PE LW/MM and per SWDGE
        # DMA, 2 per HWDGE DMA), and Tile's wait assignment is per-proc
        # minimal but not transitive. So every cross-engine dependency is
        # absorbed by a dedicated cheap "carrier" op on the consuming engine,
        # with explicit ordering edges so the scheduler keeps each carrier
        # ahead of its dependents and every instruction introduces at most
        # one new wait.
        def order(after, before):
            add_dep_helper(after.ins, before.ins, sync=False, reason="carrier order")

        for t in range(TT):
            xhi = xpool.tile([128, KT, 128], bf16, tag="xhi")
            # POOL carrier chain, one wait each: gen-2 x-load DMA (its
            # lane sem would otherwise ride the new DMA as a WAW wait) and
            # gen-2 matmul (x slot reader), before the x-slot rewrite.
            ms1 = nc.gpsimd.memset(dummy[:, 3 * t : 3 * t + 1], 0)
            ms3 = nc.gpsimd.memset(dummy[:, 3 * t + 2 : 3 * t + 3], 0)
            order(ms3, ms1)
            if t >= 2:
                add_dep_helper(
                    ms1.ins, x_dmas[t - 2].ins, reason="x WAW lane via carrier"
                )
                add_dep_helper(
                    ms3.ins,
                    last_mms[t - 2].ins,
                    reason="x slot reuse gated on POOL carrier",
                )
            d2 = nc.gpsimd.dma_start(xhi[:], xhi_d[t])
            order(d2, ms3)
            x_dmas.append(d2)

            ps = pspool.tile([128, NB, 512], f32)
            # PE carrier: guard LDWEIGHTS absorbing the psum-slot-free (DVE)
            # wait so the first real matmul only waits on PE.
            guard = nc.tensor.ldweights(w_sb[:, 0, :128])
            if t >= 2:
                add_dep_helper(
                    guard.ins,
                    psum_readers[t - 2].ins,
                    reason="psum slot reuse gated on guard ldweights",
                )
            first_mm = None
            for k in range(KT):
                for j in range(NB):
                    mm = nc.tensor.matmul(
                        ps[:, j, :],
                        xhi[:, k, :],
                        w_sb[:, k, j * 512 : (j + 1) * 512],
                        start=(k == 0),
                        stop=(k == KT - 1),
                    )
                    if first_mm is None:
                        first_mm = mm
            order(first_mm, guard)
            last_mms.append(mm)

            ob = opool.tile([128, NB, 512], f32)
            # DVE carriers: absorb the ob-slot WAR deps (gen-2 out-store DMA
            # and gen-2 POOL scratch copy) ahead of the scale-mult.
            c1 = nc.vector.tensor_copy(dveA[:, t : t + 1], scale_sb[:1, 0, :1])
            c2 = nc.vector.tensor_copy(dveB[:, t : t + 1], scale_sb[:1, 0, :1])
            if t >= 2:
                add_dep_helper(
                    c1.ins, out_dmas[t - 2].ins, reason="ob reuse vs out dma"
                )
                add_dep_helper(
                    c2.ins, out_copies[t - 2].ins, reason="ob reuse vs pool copy"
                )
            mult = nc.vector.tensor_tensor(
                ob[:], ps[:], scale_sb[:], mybir.AluOpType.mult
            )
            order(mult, c1)
            order(mult, c2)
            psum_readers.append(mult)
            adds.append(
                nc.vector.tensor_tensor(ob[:], ob[:], bias_sb[:], mybir.AluOpType.add)
            )
            # POOL carrier: RAW on ob -> absorbs the DVE wait ahead of the
            # out-store.
            cp = nc.gpsimd.tensor_copy(scratch[:, t : t + 1], ob[:1, 0, :1])
            od = nc.gpsimd.dma_start(out_d[t], ob[:])
            order(od, cp)
            out_copies.append(cp)
            out_dmas.append(od)

        # Tail carriers: SP nops, one wait each, observing every outstanding
        # sem (PE, DVE, Pool, all SWDGE lanes, preamble HWDGE lanes) so the
        # kernel-tail SP drain doesn't exceed its sync-wait slots.
        tail_deps = [
            last_mms[-1],
            adds[-1],
            out_copies[-1],
            w_dma,
            scale_dma,
            bias_dma,
        ]
        # 2 SWDGE DMAs per iteration striped over 8 global SWDGE sems ->
        # the last 4 iterations cover all 8.
        for i in (1, 2, 3, 4):
            tail_deps += [out_dmas[-i], x_dmas[-i]]
        for i, dep in enumerate(tail_deps):
            nop = nc.engines[mybir.EngineType.SP].nop(
                nofuse=True, hint=f"tail_carrier_{i}"
            )
            add_dep_helper(nop.ins, dep.ins, reason="tail drain carrier")

    return nc


def kernel(x, weight_int8, scale, bias):
    global _last_results
    from concourse.bass_utils import run_bass_kernel_spmd

    x = np.asarray(x)
    weight_int8 = np.asarray(weight_int8)
    scale = np.asarray(scale, dtype=np.float32)
    bias = np.asarray(bias, dtype=np.float32)

    # x^T [IN, TOK] rounded to bf16, tiled to [TT, 128p(IN), KT, 128(tok)]
    xT = np.ascontiguousarray(x.reshape(TOK, IN).astype(np.float32).T)
    x_hi = xT.astype(BF16)
    x_hi = np.ascontiguousarray(
        x_hi.reshape(KT, 128, TT, 128).transpose(2, 1, 0, 3)
    )

    in_maps = []
    for c in range(NCORES):
        wc = weight_int8[c * OSH : (c + 1) * OSH].astype(np.float32).astype(BF16)
        # w^T [IN, OSH] tiled to [128p(IN), KT, OSH]
        wp = np.ascontiguousarray(wc.T.reshape(KT, 128, OSH).transpose(1, 0, 2))
        sc = np.ascontiguousarray(
            np.broadcast_to(scale[c * OSH : (c + 1) * OSH], (128, OSH))
        ).reshape(128, NB, 512)
        bc = np.ascontiguousarray(
            np.broadcast_to(bias[c * OSH : (c + 1) * OSH], (128, OSH))
        ).reshape(128, NB, 512)
        in_maps.append({"xhi": x_hi, "w": wp, "scale": sc, "bias": bc})

    nc = _build_program()
    trace = bool(os.environ.get("KERNEL_TRACE"))
    kwargs = {}
    if trace:
        # Local-only profiling: stub the bucket upload and install the axon
        # NTFF hook (the image's antenv stub lacks axon_hooks).
        import sys
        import types

        from concourse import bass_utils as _bu

        _bu.upload_artifacts = lambda tmpdir: "local://" + tmpdir
        if "antenv.axon_hooks" not in sys.modules:
            import antenv

            mod = types.ModuleType("antenv.axon_hooks")
            _holder = [None]
            mod.set_axon_ntff_profile_hook = lambda h: _holder.__setitem__(0, h)
            mod.get_axon_ntff_profile_hook = lambda: _holder[0]
            antenv.axon_hooks = mod
            sys.modules["antenv.axon_hooks"] = mod
        from antenv.axon_hooks import (
            get_axon_ntff_profile_hook,
            set_axon_ntff_profile_hook,
        )

        if get_axon_ntff_profile_hook() is None:
            from trn_agent_boot.trn_boot import _ntff_profile_via_ctypes

            set_axon_ntff_profile_hook(
                _ntff_profile_via_ctypes(
                    os.environ.get("PJRT_LIBRARY_PATH", "/opt/axon/libaxon_pjrt.so")
                )
            )
        tmpdir = os.environ.get("KERNEL_TRACE_DIR")
        if tmpdir:
            os.makedirs(tmpdir, exist_ok=True)
            kwargs["tmpdir"] = tmpdir

    res = run_bass_kernel_spmd(
        nc,
        in_maps,
        list(range(NCORES)),
        trace=trace,
        **kwargs,
    )
    _last_results = res

    parts = [res.results[c]["out"].reshape(TOK, OSH) for c in range(NCORES)]
    return np.concatenate(parts, axis=1).reshape(B, S, OUT)



# revision 18
# speedup vs baseline: 2.0085x; 1.0254x over previous
"""CompressedLinear on 8 Trainium2 NeuronCores.

out[b,s,o] = sum_i x[b,s,i] * (w_int8[o,i] * scale[o]) + bias[o]
  x: [4, 2048, 4096] f32, w_int8: [16384, 4096] int32 (codes in [-64,63]),
  scale/bias: [16384] f32 -> out: [4, 2048, 16384] f32

Strategy (tensor-parallel over out_features):
  - Each of the 8 cores owns a 2048-row slice of W/scale/bias and computes
    out[:, :, c*2048:(c+1)*2048]; x is replicated.
  - x (f32) and w (codes*scale) are rounded to bf16 host-side; a single
    bf16 matmul pass accumulates in f32 PSUM -> ~2.3e-3 relative error
    (tolerance is 2e-2) at 1x bf16 matmul cost. This is the PE roofline:
    fp8 DoubleRow would need >=3 digit-product passes at 2x rate for this
    precision (worse), and f32 runs at 1/4 rate.
  - w is DMA'd in 32 per-k-tile chunks so the t=0 matmuls start ~5us in
    and ride the w stream instead of waiting ~50us for the full 16.8MB.
  - Per core loop: stationary operand = 128-token column block of x^T,
    moving operand = w^T; PSUM holds [128 tokens, 4x512 outfeat]; 32 k-tiles
    x 4 banks = 128 matmuls per token tile, then one bias-add epilogue on
    DVE (reads PSUM, writes SBUF) and a DMA store.

All data layout transforms (transpose, hi/lo split, int8->bf16 cast,
scale/bias broadcast) are host-side numpy; gather is a concat.
"""

import os

import numpy as np
import ml_dtypes

BF16 = ml_dtypes.bfloat16

OUT, IN = 16384, 4096
B, S = 4, 2048
TOK = B * S            # 8192 tokens
NCORES = 8
OSH = OUT // NCORES    # 2048 out-features per core
KT = IN // 128         # 32 k-tiles
TT = TOK // 128        # 64 token tiles
NB = OSH // 512        # 4 psum banks per token tile

_last_results = None   # BassKernelResults of the most recent run (for test.py)


def _build_program():
    from contextlib import ExitStack

    import concourse.bass as bass
    import concourse.tile as tile
    from concourse import mybir

    f32 = mybir.dt.float32
    bf16 = mybir.dt.bfloat16

    nc = bass.Bass()
    xhi_d = nc.declare_dram_parameter("xhi", [TT, 128, KT, 128], bf16, isOutput=False)
    w_d = nc.declare_dram_parameter("w", [128, KT, OSH], bf16, isOutput=False)
    bias_d = nc.declare_dram_parameter("bias", [128, NB, 512], f32, isOutput=False)
    out_d = nc.declare_dram_parameter("out", [TT, 128, NB, 512], f32, isOutput=True)

    from concourse.tile import add_dep_helper

    with tile.TileContext(nc) as tc, ExitStack() as ctx:
        wpool = ctx.enter_context(tc.tile_pool(name="w", bufs=1))
        cpool = ctx.enter_context(tc.tile_pool(name="consts", bufs=1))
        xpool = ctx.enter_context(tc.tile_pool(name="x", bufs=2))
        opool = ctx.enter_context(tc.tile_pool(name="o", bufs=2))
        pspool = ctx.enter_context(tc.tile_pool(name="ps", bufs=2, space="PSUM"))

        # bias first, then w chunked per k-tile so the first matmuls can
        # start as soon as chunk 0 lands (~5us) instead of after the whole
        # 16.8MB w load (~50us); the t=0 k-loop rides the w DMA stream.
        bias_sb = cpool.tile([128, NB, 512], f32, tag="bias")
        bias_dma = nc.sync.dma_start(bias_sb[:], bias_d[:])
        w_sb = wpool.tile([128, KT, OSH], bf16)
        w_dmas = [
            nc.sync.dma_start(w_sb[:, k, :], w_d[:, k, :]) for k in range(KT)
        ]

        # Per-iteration disjoint scratch columns -> the carrier ops carry no
        # WAW deps of their own.
        scratch = cpool.tile([1, TT], f32, tag="scratch")
        dummy = cpool.tile([1, 3 * TT], f32, tag="dummy")
        dveA = cpool.tile([1, TT], f32, tag="dveA")
        dveB = cpool.tile([1, TT], f32, tag="dveB")
        dveC = cpool.tile([1, TT], f32, tag="dveC")
        # Preamble DVE carrier: observe the bias const load on DVE so no
        # steady-state DVE op pairs a DMAHW wait with another wait.
        pre = cpool.tile([1, 2], f32, tag="pre")
        nc.vector.tensor_copy(pre[:, 0:1], bias_sb[:1, 0, :1])

        psum_readers = []  # the bias-add (last psum reader) per iteration
        last_mms = []  # final matmul per iteration
        out_dmas = []
        out_copies = []
        x_dmas = []
        adds = []

        # Hardware sync-wait slots are tiny (1 per PE LW/MM and per SWDGE
        # DMA, 2 per HWDGE DMA), and Tile's wait assignment is per-proc
        # minimal but not transitive. So every cross-engine dependency is
        # absorbed by a dedicated cheap "carrier" op on the consuming engine,
        # with explicit ordering edges so the scheduler keeps each carrier
        # ahead of its dependents and every instruction introduces at most
        # one new wait.
        def order(after, before):
            add_dep_helper(after.ins, before.ins, sync=False, reason="carrier order")

        for t in range(TT):
            xhi = xpool.tile([128, KT, 128], bf16, tag="xhi")
            # POOL carrier chain, one wait each: gen-2 x-load DMA (its
            # lane sem would otherwise ride the new DMA as a WAW wait) and
            # gen-2 matmul (x slot reader), before the x-slot rewrite.
            ms1 = nc.gpsimd.memset(dummy[:, 3 * t : 3 * t + 1], 0)
            ms3 = nc.gpsimd.memset(dummy[:, 3 * t + 2 : 3 * t + 3], 0)
            order(ms3, ms1)
            if t >= 2:
                add_dep_helper(
                    ms1.ins, x_dmas[t - 2].ins, reason="x WAW lane via carrier"
                )
                add_dep_helper(
                    ms3.ins,
                    last_mms[t - 2].ins,
                    reason="x slot reuse gated on POOL carrier",
                )
            d2 = nc.gpsimd.dma_start(xhi[:], xhi_d[t])
            order(d2, ms3)
            x_dmas.append(d2)

            ps = pspool.tile([128, NB, 512], f32)
            # PE carrier: guard LDWEIGHTS absorbing the psum-slot-free (DVE)
            # wait so the first real matmul only waits on PE.
            guard = nc.tensor.ldweights(w_sb[:, 0, :128])
            if t >= 2:
                add_dep_helper(
                    guard.ins,
                    psum_readers[t - 2].ins,
                    reason="psum slot reuse gated on guard ldweights",
                )
            first_mm = None
            for k in range(KT):
                for j in range(NB):
                    mm = nc.tensor.matmul(
                        ps[:, j, :],
                        xhi[:, k, :],
                        w_sb[:, k, j * 512 : (j + 1) * 512],
                        start=(k == 0),
                        stop=(k == KT - 1),
                    )
                    if first_mm is None:
                        first_mm = mm
            order(first_mm, guard)
            last_mms.append(mm)

            ob = opool.tile([128, NB, 512], f32)
            # DVE carriers: absorb the ob-slot WAR deps (gen-2 out-store DMA
            # and gen-2 POOL scratch copy) ahead of the bias-add. scale is
            # folded into w host-side, so the epilogue is one DVE op.
            c1 = nc.vector.tensor_copy(dveA[:, t : t + 1], bias_sb[:1, 0, :1])
            c2 = nc.vector.tensor_copy(dveB[:, t : t + 1], bias_sb[:1, 0, :1])
            if t >= 2:
                add_dep_helper(
                    c1.ins, out_dmas[t - 2].ins, reason="ob reuse vs out dma"
                )
                add_dep_helper(
                    c2.ins, out_copies[t - 2].ins, reason="ob reuse vs pool copy"
                )
            # 1-element DVE carrier reading the last-written psum bank: it
            # absorbs the PE-sem wait so the full-size add carries only its
            # own-engine wait (TT has a single sync-wait slot).
            pc = nc.vector.tensor_copy(dveC[:, t : t + 1], ps[:1, NB - 1, :1])
            add = nc.vector.tensor_tensor(
                ob[:], ps[:], bias_sb[:], mybir.AluOpType.add
            )
            order(add, pc)
            order(add, c1)
            order(add, c2)
            psum_readers.append(add)
            adds.append(add)
            # POOL carrier: RAW on ob -> absorbs the DVE wait ahead of the
            # out-store.
            cp = nc.gpsimd.tensor_copy(scratch[:, t : t + 1], ob[:1, 0, :1])
            od = nc.gpsimd.dma_start(out_d[t], ob[:])
            order(od, cp)
            out_copies.append(cp)
            out_dmas.append(od)

        # Tail carriers: SP nops, one wait each, observing every outstanding
        # sem (PE, DVE, Pool, all SWDGE lanes, preamble HWDGE lanes) so the
        # kernel-tail SP drain doesn't exceed its sync-wait slots.
        tail_deps = [
            last_mms[-1],
            adds[-1],
            out_copies[-1],
            bias_dma,
        ]
        # HWDGE DMAs stripe over 8 sems -> the last 8 w chunks cover all.
        tail_deps += w_dmas[-8:]
        # 2 SWDGE DMAs per iteration striped over 8 global SWDGE sems ->
        # the last 4 iterations cover all 8.
        for i in (1, 2, 3, 4):
            tail_deps += [out_dmas[-i], x_dmas[-i]]
        for i, dep in enumerate(tail_deps):
            nop = nc.engines[mybir.EngineType.SP].nop(
                nofuse=True, hint=f"tail_carrier_{i}"
            )
            add_dep_helper(nop.ins, dep.ins, reason="tail drain carrier")

    return nc


def kernel(x, weight_int8, scale, bias):
    global _last_results
    from concourse.bass_utils import run_bass_kernel_spmd

    x = np.asarray(x)
    weight_int8 = np.asarray(weight_int8)
    scale = np.asarray(scale, dtype=np.float32)
    bias = np.asarray(bias, dtype=np.float32)

    # x^T [IN, TOK] rounded to bf16, tiled to [TT, 128p(IN), KT, 128(tok)]
    xT = np.ascontiguousarray(x.reshape(TOK, IN).astype(np.float32).T)
    x_hi = xT.astype(BF16)
    x_hi = np.ascontiguousarray(
        x_hi.reshape(KT, 128, TT, 128).transpose(2, 1, 0, 3)
    )

    in_maps = []
    for c in range(NCORES):
        # scale folded into w host-side (bf16 rounding of codes*scale adds
        # ~1e-3 rel err; total ~2.3e-3 vs the 2e-2 gate)
        wc = (
            weight_int8[c * OSH : (c + 1) * OSH].astype(np.float32)
            * scale[c * OSH : (c + 1) * OSH, None]
        ).astype(BF16)
        # w^T [IN, OSH] tiled to [128p(IN), KT, OSH]
        wp = np.ascontiguousarray(wc.T.reshape(KT, 128, OSH).transpose(1, 0, 2))
        bc = np.ascontiguousarray(
            np.broadcast_to(bias[c * OSH : (c + 1) * OSH], (128, OSH))
        ).reshape(128, NB, 512)
        in_maps.append({"xhi": x_hi, "w": wp, "bias": bc})

    nc = _build_program()
    trace = bool(os.environ.get("KERNEL_TRACE"))
    kwargs = {}
    if trace:
        # Local-only profiling: stub the bucket upload and install the axon
        # NTFF hook (the image's antenv stub lacks axon_hooks).
        import sys
        import types

        from concourse import bass_utils as _bu

        _bu.upload_artifacts = lambda tmpdir: "local://" + tmpdir
        if "antenv.axon_hooks" not in sys.modules:
            import antenv

            mod = types.ModuleType("antenv.axon_hooks")
            _holder = [None]
            mod.set_axon_ntff_profile_hook = lambda h: _holder.__setitem__(0, h)
            mod.get_axon_ntff_profile_hook = lambda: _holder[0]
            antenv.axon_hooks = mod
            sys.modules["antenv.axon_hooks"] = mod
        from antenv.axon_hooks import (
            get_axon_ntff_profile_hook,
            set_axon_ntff_profile_hook,
        )

        if get_axon_ntff_profile_hook() is None:
            from trn_agent_boot.trn_boot import _ntff_profile_via_ctypes

            set_axon_ntff_profile_hook(
                _ntff_profile_via_ctypes(
                    os.environ.get("PJRT_LIBRARY_PATH", "/opt/axon/libaxon_pjrt.so")
                )
            )
        tmpdir = os.environ.get("KERNEL_TRACE_DIR")
        if tmpdir:
            os.makedirs(tmpdir, exist_ok=True)
            kwargs["tmpdir"] = tmpdir

    res = run_bass_kernel_spmd(
        nc,
        in_maps,
        list(range(NCORES)),
        trace=trace,
        **kwargs,
    )
    _last_results = res

    parts = [res.results[c]["out"].reshape(TOK, OSH) for c in range(NCORES)]
    return np.concatenate(parts, axis=1).reshape(B, S, OUT)



# revision 24
# speedup vs baseline: 2.0095x; 1.0005x over previous
"""CompressedLinear on 8 Trainium2 NeuronCores.

out[b,s,o] = sum_i x[b,s,i] * (w_int8[o,i] * scale[o]) + bias[o]
  x: [4, 2048, 4096] f32, w_int8: [16384, 4096] int32 (codes in [-64,63]),
  scale/bias: [16384] f32 -> out: [4, 2048, 16384] f32

Strategy (tensor-parallel over out_features):
  - Each of the 8 cores owns a 2048-row slice of W/scale/bias and computes
    out[:, :, c*2048:(c+1)*2048]; x is replicated.
  - x (f32) and w (codes*scale) are rounded to bf16 host-side; a single
    bf16 matmul pass accumulates in f32 PSUM -> ~2.3e-3 relative error
    (tolerance is 2e-2) at 1x bf16 matmul cost. This is the PE roofline:
    fp8 DoubleRow would need >=3 digit-product passes at 2x rate for this
    precision (worse), and f32 runs at 1/4 rate.
  - w is DMA'd in 32 per-k-tile chunks so the t=0 matmuls start ~5us in
    and ride the w stream instead of waiting ~50us for the full 16.8MB.
  - Per core loop: stationary operand = 128-token column block of x^T,
    moving operand = w^T; PSUM holds [128 tokens, 4x512 outfeat]; 32 k-tiles
    x 4 banks = 128 matmuls per token tile, then one bias-add epilogue on
    DVE (reads PSUM, writes SBUF) and a DMA store.

All data layout transforms (transpose, hi/lo split, int8->bf16 cast,
scale/bias broadcast) are host-side numpy; gather is a concat.
"""

import os

import numpy as np
import ml_dtypes

BF16 = ml_dtypes.bfloat16

OUT, IN = 16384, 4096
B, S = 4, 2048
TOK = B * S            # 8192 tokens
NCORES = 8
OSH = OUT // NCORES    # 2048 out-features per core
KT = IN // 128         # 32 k-tiles
TT = TOK // 128        # 64 token tiles
NB = OSH // 512        # 4 psum banks per token tile

_last_results = None   # BassKernelResults of the most recent run (for test.py)


def _build_program():
    from contextlib import ExitStack

    import concourse.bass as bass
    import concourse.tile as tile
    from concourse import mybir

    f32 = mybir.dt.float32
    bf16 = mybir.dt.bfloat16

    nc = bass.Bass()
    xhi_d = nc.declare_dram_parameter("xhi", [TT, 128, KT, 128], bf16, isOutput=False)
    w_d = nc.declare_dram_parameter("w", [128, KT, OSH], bf16, isOutput=False)
    bias_d = nc.declare_dram_parameter("bias", [128, NB, 512], f32, isOutput=False)
    out_d = nc.declare_dram_parameter("out", [TT, 128, NB, 512], f32, isOutput=True)

    from concourse.tile import add_dep_helper

    with tile.TileContext(nc) as tc, ExitStack() as ctx:
        wpool = ctx.enter_context(tc.tile_pool(name="w", bufs=1))
        cpool = ctx.enter_context(tc.tile_pool(name="consts", bufs=1))
        xpool = ctx.enter_context(tc.tile_pool(name="x", bufs=2))
        opool = ctx.enter_context(tc.tile_pool(name="o", bufs=2))
        pspool = ctx.enter_context(tc.tile_pool(name="ps", bufs=2, space="PSUM"))

        # w chunked per k-tile so the first matmuls can start as soon as
        # chunk 0 lands (~9us) instead of after the whole 16.8MB w load
        # (~50us); the t=0 k-loop rides the w DMA stream. bias goes last on
        # the ring -- it's only needed by the first epilogue (~46us in).
        w_sb = wpool.tile([128, KT, OSH], bf16)
        w_dmas = [
            nc.sync.dma_start(w_sb[:, k, :], w_d[:, k, :]) for k in range(KT)
        ]
        bias_sb = cpool.tile([128, NB, 512], f32, tag="bias")
        bias_dma = nc.sync.dma_start(bias_sb[:], bias_d[:])
        hwdge_all = w_dmas + [bias_dma]

        # Per-iteration disjoint scratch columns -> the carrier ops carry no
        # WAW deps of their own.
        scratch = cpool.tile([1, TT], f32, tag="scratch")
        dummy = cpool.tile([1, 3 * TT], f32, tag="dummy")
        dveA = cpool.tile([1, TT], f32, tag="dveA")
        dveB = cpool.tile([1, TT], f32, tag="dveB")
        dveC = cpool.tile([1, TT], f32, tag="dveC")
        # Preamble DVE carrier: observe the bias const load on DVE so no
        # steady-state DVE op pairs a DMAHW wait with another wait.
        pre = cpool.tile([1, 2], f32, tag="pre")
        nc.vector.tensor_copy(pre[:, 0:1], bias_sb[:1, 0, :1])

        psum_readers = []  # the bias-add (last psum reader) per iteration
        last_mms = []  # final matmul per iteration
        out_dmas = []
        out_copies = []
        x_dmas = []
        adds = []
        swdge_all = []  # every SWDGE DMA in emission order (tail coverage)

        # Hardware sync-wait slots are tiny (1 per PE LW/MM and per SWDGE
        # DMA, 2 per HWDGE DMA), and Tile's wait assignment is per-proc
        # minimal but not transitive. So every cross-engine dependency is
        # absorbed by a dedicated cheap "carrier" op on the consuming engine,
        # with explicit ordering edges so the scheduler keeps each carrier
        # ahead of its dependents and every instruction introduces at most
        # one new wait.
        def order(after, before):
            add_dep_helper(after.ins, before.ins, sync=False, reason="carrier order")

        for t in range(TT):
            xhi = xpool.tile([128, KT, 128], bf16, tag="xhi")
            # POOL carrier chain, one wait each: gen-2 x-load DMA (its
            # lane sem would otherwise ride the new DMA as a WAW wait) and
            # gen-2 matmul (x slot reader), before the x-slot rewrite.
            ms1 = nc.gpsimd.memset(dummy[:, 3 * t : 3 * t + 1], 0)
            ms3 = nc.gpsimd.memset(dummy[:, 3 * t + 2 : 3 * t + 3], 0)
            order(ms3, ms1)
            if t >= 2:
                prev = x_dmas[t - 2]
                add_dep_helper(
                    ms1.ins, prev[-1].ins, reason="x WAW lane via carrier"
                )
                # distinct, otherwise-unused columns (3s+1 of tiles 0..2) --
                # sharing one column creates WAW deps that Tile emits as
                # Pool self-sem waits, overflowing the 1-slot limit.
                for s, sub in enumerate(prev[:-1]):
                    msx = nc.gpsimd.memset(dummy[:, 3 * s + 1 : 3 * s + 2], 0)
                    add_dep_helper(
                        msx.ins, sub.ins, reason="x WAW lane via carrier"
                    )
                    order(ms3, msx)
                add_dep_helper(
                    ms3.ins,
                    last_mms[t - 2].ins,
                    reason="x slot reuse gated on POOL carrier",
                )
            if t == 0:
                # 4 sub-DMAs: the k=0..7 slice (256KB) lands ~1us in, so the
                # first matmuls gate at ~10us instead of the full 1MB x tile.
                ds = []
                for s in range(4):
                    sub = nc.gpsimd.dma_start(
                        xhi[:, 8 * s : 8 * (s + 1), :],
                        xhi_d[0][:, 8 * s : 8 * (s + 1), :],
                    )
                    order(sub, ms3)
                    ds.append(sub)
                x_dmas.append(ds)
                swdge_all += ds
            else:
                d2 = nc.gpsimd.dma_start(xhi[:], xhi_d[t])
                order(d2, ms3)
                x_dmas.append([d2])
                swdge_all.append(d2)

            ps = pspool.tile([128, NB, 512], f32)
            # PE carrier: guard LDWEIGHTS absorbing the psum-slot-free (DVE)
            # wait so the first real matmul only waits on PE.
            guard = nc.tensor.ldweights(w_sb[:, 0, :128])
            if t >= 2:
                add_dep_helper(
                    guard.ins,
                    psum_readers[t - 2].ins,
                    reason="psum slot reuse gated on guard ldweights",
                )
            first_mm = None
            for k in range(KT):
                for j in range(NB):
                    mm = nc.tensor.matmul(
                        ps[:, j, :],
                        xhi[:, k, :],
                        w_sb[:, k, j * 512 : (j + 1) * 512],
                        start=(k == 0),
                        stop=(k == KT - 1),
                    )
                    if first_mm is None:
                        first_mm = mm
            order(first_mm, guard)
            last_mms.append(mm)

            ob = opool.tile([128, NB, 512], f32)
            # DVE carriers: absorb the ob-slot WAR deps (gen-2 out-store DMA
            # and gen-2 POOL scratch copy) ahead of the bias-add. scale is
            # folded into w host-side, so the epilogue is one DVE op.
            c1 = nc.vector.tensor_copy(dveA[:, t : t + 1], bias_sb[:1, 0, :1])
            c2 = nc.vector.tensor_copy(dveB[:, t : t + 1], bias_sb[:1, 0, :1])
            if t >= 2:
                add_dep_helper(
                    c1.ins, out_dmas[t - 2].ins, reason="ob reuse vs out dma"
                )
                add_dep_helper(
                    c2.ins, out_copies[t - 2].ins, reason="ob reuse vs pool copy"
                )
            if t < TT - 1:
                # 1-element DVE carrier reading the last-written psum bank:
                # it absorbs the PE-sem wait so the full-size add carries
                # only its own-engine wait (TT has a single sync-wait slot).
                pc = nc.vector.tensor_copy(
                    dveC[:, t : t + 1], ps[:1, NB - 1, :1]
                )
                add = nc.vector.tensor_tensor(
                    ob[:], ps[:], bias_sb[:], mybir.AluOpType.add
                )
                order(add, pc)
                order(add, c1)
                order(add, c2)
                psum_readers.append(add)
                adds.append(add)
                # POOL carrier: RAW on ob -> absorbs the DVE wait ahead of
                # the out-store.
                cp = nc.gpsimd.tensor_copy(scratch[:, t : t + 1], ob[:1, 0, :1])
                od = nc.gpsimd.dma_start(out_d[t], ob[:])
                order(od, cp)
                out_copies.append(cp)
                out_dmas.append(od)
                swdge_all.append(od)
            else:
                # Last tile: split the epilogue into two bank-halves so the
                # final add/store overlap the last matmuls instead of
                # serializing the whole 2.3us add + 2.9us DMA after them.
                pc01 = nc.vector.tensor_copy(dveC[:, t : t + 1], ps[:1, 1, :1])
                add01 = nc.vector.tensor_tensor(
                    ob[:, 0:2, :], ps[:, 0:2, :], bias_sb[:, 0:2, :],
                    mybir.AluOpType.add,
                )
                order(add01, pc01)
                order(add01, c1)
                order(add01, c2)
                pc23 = nc.vector.tensor_copy(pre[:, 1:2], ps[:1, 3, :1])
                order(pc23, add01)
                add23 = nc.vector.tensor_tensor(
                    ob[:, 2:4, :], ps[:, 2:4, :], bias_sb[:, 2:4, :],
                    mybir.AluOpType.add,
                )
                order(add23, pc23)
                psum_readers.append(add23)
                adds.append(add23)
                cp_a = nc.gpsimd.tensor_copy(
                    scratch[:, t : t + 1], ob[:1, 0, :1]
                )
                od_a = nc.gpsimd.dma_start(out_d[t][:, 0:2, :], ob[:, 0:2, :])
                order(od_a, cp_a)
                cp_b = nc.gpsimd.tensor_copy(
                    dummy[:, 3 * t + 1 : 3 * t + 2], ob[:1, 2, :1]
                )
                order(cp_b, od_a)
                od_b = nc.gpsimd.dma_start(out_d[t][:, 2:4, :], ob[:, 2:4, :])
                order(od_b, cp_b)
                out_copies.append(cp_b)
                out_dmas.append(od_b)
                swdge_all += [od_a, od_b]

        # Tail carriers: SP nops, one wait each, observing every outstanding
        # sem (PE, DVE, Pool, all SWDGE lanes, preamble HWDGE lanes) so the
        # kernel-tail SP drain doesn't exceed its sync-wait slots.
        tail_deps = [
            last_mms[-1],
            adds[-1],
            out_copies[-1],
        ]
        # HWDGE / SWDGE DMAs stripe over 8 sems each -> covering the last
        # 8 (plus slack) observes every lane's final value.
        tail_deps += hwdge_all[-8:]
        tail_deps += swdge_all[-10:]
        for i, dep in enumerate(tail_deps):
            nop = nc.engines[mybir.EngineType.SP].nop(
                nofuse=True, hint=f"tail_carrier_{i}"
            )
            add_dep_helper(nop.ins, dep.ins, reason="tail drain carrier")

    return nc


def kernel(x, weight_int8, scale, bias):
    global _last_results
    from concourse.bass_utils import run_bass_kernel_spmd

    x = np.asarray(x)
    weight_int8 = np.asarray(weight_int8)
    scale = np.asarray(scale, dtype=np.float32)
    bias = np.asarray(bias, dtype=np.float32)

    # x^T [IN, TOK] rounded to bf16, tiled to [TT, 128p(IN), KT, 128(tok)]
    xT = np.ascontiguousarray(x.reshape(TOK, IN).astype(np.float32).T)
    x_hi = xT.astype(BF16)
    x_hi = np.ascontiguousarray(
        x_hi.reshape(KT, 128, TT, 128).transpose(2, 1, 0, 3)
    )

    in_maps = []
    for c in range(NCORES):
        # scale folded into w host-side (bf16 rounding of codes*scale adds
        # ~1e-3 rel err; total ~2.3e-3 vs the 2e-2 gate)
        wc = (
            weight_int8[c * OSH : (c + 1) * OSH].astype(np.float32)
            * scale[c * OSH : (c + 1) * OSH, None]
        ).astype(BF16)
        # w^T [IN, OSH] tiled to [128p(IN), KT, OSH]
        wp = np.ascontiguousarray(wc.T.reshape(KT, 128, OSH).transpose(1, 0, 2))
        bc = np.ascontiguousarray(
            np.broadcast_to(bias[c * OSH : (c + 1) * OSH], (128, OSH))
        ).reshape(128, NB, 512)
        in_maps.append({"xhi": x_hi, "w": wp, "bias": bc})

    nc = _build_program()
    trace = bool(os.environ.get("KERNEL_TRACE"))
    kwargs = {}
    if trace:
        # Local-only profiling: stub the bucket upload and install the axon
        # NTFF hook (the image's antenv stub lacks axon_hooks).
        import sys
        import types

        from concourse import bass_utils as _bu

        _bu.upload_artifacts = lambda tmpdir: "local://" + tmpdir
        if "antenv.axon_hooks" not in sys.modules:
            import antenv

            mod = types.ModuleType("antenv.axon_hooks")
            _holder = [None]
            mod.set_axon_ntff_profile_hook = lambda h: _holder.__setitem__(0, h)
            mod.get_axon_ntff_profile_hook = lambda: _holder[0]
            antenv.axon_hooks = mod
            sys.modules["antenv.axon_hooks"] = mod
        from antenv.axon_hooks import (
            get_axon_ntff_profile_hook,
            set_axon_ntff_profile_hook,
        )

        if get_axon_ntff_profile_hook() is None:
            from trn_agent_boot.trn_boot import _ntff_profile_via_ctypes

            set_axon_ntff_profile_hook(
                _ntff_profile_via_ctypes(
                    os.environ.get("PJRT_LIBRARY_PATH", "/opt/axon/libaxon_pjrt.so")
                )
            )
        tmpdir = os.environ.get("KERNEL_TRACE_DIR")
        if tmpdir:
            os.makedirs(tmpdir, exist_ok=True)
            kwargs["tmpdir"] = tmpdir

    res = run_bass_kernel_spmd(
        nc,
        in_maps,
        list(range(NCORES)),
        trace=trace,
        **kwargs,
    )
    _last_results = res

    parts = [res.results[c]["out"].reshape(TOK, OSH) for c in range(NCORES)]
    return np.concatenate(parts, axis=1).reshape(B, S, OUT)

